# revision 33
# baseline (speedup 1.0000x reference)
# Trainium2 Bass kernel for BertCrf4Tecs:
#   em = sigmoid(hidden @ W + b) reshaped [B, T, K]
#   log_likelihood = crf_sequence_score(em, target, trans) - crf_log_norm(em, trans)
#   decode = viterbi_decode(em, trans)
#
# Data-parallel over batch on 8 NeuronCores (1024 sequences per core).
# Per-core structure (emission order is interleaved so the PE/ACT work of
# the emission matmuls hides under the DVE/GPSIMD-bound recurrence):
#   group n = 0..5:
#     A-chunk n: batch-major z = hidden@W + b for 16 t's (fp32 matmul,
#       exact), em = 0.5 + 0.5*tanh(z/2) via ACT tanh (more accurate than
#       the Sigmoid table), unary gold score folded in via iota/is_eq
#       masks, em chunk spilled to DRAM.
#     B-tiles 4n..4n+3: time-major z^T (fp32r matmul - the log-norm path
#       tolerates reduced precision), g^T = exp(em^T) to DRAM.
#     steps t = 16n-1 .. 16n+14:
#       CRF forward in the exp domain: p_{t+1} = (E^T p_t) * g_{t+1}, E as
#         32x32 stationary blocks on a cycling partition block so the PSUM
#         output block matches the g^T tile row block; rescale by 1/sum
#         every 8 steps, accumulate ln into lnacc.
#       Viterbi forward, exact and bit-faithful to the reference
#         recurrence, as 4 independent 2-btile chains so the DVE and
#         GPSIMD pipelines don't serialize:
#           tmp[b, k', k] = v[b, k] + T[k, k']  (stride-0 broadcast APs;
#                                                1 chain on DVE, 3 on GPSIMD)
#           v' = max_k tmp (segmented reduce_max, DVE) + em[t+1]
#         v_t spilled to DRAM for the lazy backtrace.
#   backtrace (t = 94..0): tag_t = argmax_k(v_t[k] + T[k, tag_{t+1}]):
#     T[:, tag] gathered via one-hot(tag) PE-transpose + one-hot matmul
#     (each tile_position row-group in its own PSUM bank - different-row-
#     group accumulation groups sharing a bank crash the device), argmax
#     via is_ge mask + masked-iota reduce_min (first-index tie rule,
#     matching jnp.argmax).
#
# log_likelihood output = unary - logZ computed on device; the binary
# (transition) score is a pure function of two unmodified inputs
# (target, transitions) and is added on the host.

import os
import sys

sys.path.insert(0, "/opt/trn_rl_repo")

import numpy as np

import concourse.bacc as bacc
import concourse.tile as tile
from concourse import mybir
from concourse.bass_utils import run_bass_kernel_spmd

B, T, K, H = 8192, 96, 32, 768
NCORES = 8
BC = B // NCORES          # 1024 sequences per core
NBT = BC // 128           # 8 batch tiles of 128
NH = H // 128             # 6 contraction tiles
TK = T * K                # 3072
NTK = TK // 128           # 24 time-major row tiles (4 t's each)
NN = TK // 512            # 6 batch-major column chunks (16 t's each)
RENORM = 8                # CRF forward rescale period
BIGF = 1024.0             # argmax iota offset

AF = mybir.ActivationFunctionType
OP = mybir.AluOpType
F32 = mybir.dt.float32
F32R = mybir.dt.float32r
I32 = mybir.dt.int32

_CACHE = {}

# ---------------------------------------------------------------------------
# Custom DVE op: segmented running max of (Src0 + Src1) along the innermost
# free dim, state reset at each subdim boundary (one fused pass replaces the
# tensor_tensor add + tensor_reduce max pair of the Viterbi inner loop).
# The stock Scan node has no per-subdim reset; we emit the reset step-state
# (state' = MAX(-FLT_MAX, expr) on the first element of each segment) through
# a targeted patch of dve_spec._scan_overrides for this one scan node.
# ---------------------------------------------------------------------------
from concourse import dve_spec as _ds
from concourse import dve_ops as _do
from concourse.dve_spec import Spec as _Spec, Src0 as _Src0, Src1 as _Src1, \
    AluOp as _AluOp, lower as _dve_lower
from concourse.dve_uop import DveOpSpec as _DveOpSpec

_VITSCAN = _ds.scan(_AluOp.MAX, _Src0 + _Src1)

_orig_scan_overrides = _ds._scan_overrides


def _patched_scan_overrides(scans, node_stage):
    seed, step = _orig_scan_overrides(scans, node_stage)
    for s in scans:
        if s is _VITSCAN:
            d = node_stage[s]
            step[d] = _ds._Stage(s.op, _ds._scan_init(s), s.expr)
    return seed, step


def _ref_vitmax(in0, in1, s0, s1, imm2):
    x = in0.astype(np.float32) + in1
    return np.maximum.accumulate(x, axis=-1)


def _make_vitmax():
    if "VITMAX_SEG" in _do._SUB_OPCODE_FOR_NAME:
        return next(o for o in _do.OPS if o.name == "VITMAX_SEG")
    _ds._scan_overrides = _patched_scan_overrides
    spec = _Spec(body=_VITSCAN, reference=_ref_vitmax)
    row = _do._CUSTOM_DVE_ROW_BASE + len(_do.OPS)
    shas = {}
    for ver in ("v3", "v4"):
        try:
            uops = _dve_lower(spec, ver=ver)
            shas[ver] = _DveOpSpec(name="VITMAX_SEG", opcode=row, uops=uops,
                                   rd1_en=True).sha(ver)
        except Exception:
            pass
    op = _do.DveOp("VITMAX_SEG", spec, subdim=True, uops_sha=shas)
    _do.OPS.append(op)
    _do.CUSTOM_DVE_SPECS["VITMAX_SEG"] = spec
    _do._SUB_OPCODE_FOR_NAME["VITMAX_SEG"] = row
    return op


VITMAX = _make_vitmax()


def _build():
    nc = bacc.Bacc("TRN2", target_bir_lowering=False, debug=False)

    hT_d = nc.dram_tensor("hT", [H, BC], F32, kind="ExternalInput").ap()
    W_d = nc.dram_tensor("W", [H, TK], F32, kind="ExternalInput").ap()
    bias_d = nc.dram_tensor("bias", [1, TK], F32, kind="ExternalInput").ap()
    trans4_d = nc.dram_tensor("trans4", [128, K], F32, kind="ExternalInput").ap()
    trans4T_d = nc.dram_tensor("trans4T", [128, K], F32, kind="ExternalInput").ap()
    trep_d = nc.dram_tensor("trep", [128, K * K], F32, kind="ExternalInput").ap()
    iota_d = nc.dram_tensor("iota", [128, K], F32, kind="ExternalInput").ap()
    ident_d = nc.dram_tensor("ident", [128, 128], F32, kind="ExternalInput").ap()
    goh_d = nc.dram_tensor("goldoh", [BC, T, K], mybir.dt.bfloat16,
                           kind="ExternalInput").ap()

    dec_d = nc.dram_tensor("decode", [BC, T], I32, kind="ExternalOutput").ap()
    llp_d = nc.dram_tensor("llp", [BC], F32, kind="ExternalOutput").ap()

    with tile.TileContext(nc) as tc:
        with tc.tile_pool(name="const", bufs=1) as cpool, \
             tc.tile_pool(name="dram", bufs=1, space="DRAM") as dpool, \
             tc.tile_pool(name="persist", bufs=1) as ppool, \
             tc.tile_pool(name="pa", bufs=2) as pa, \
             tc.tile_pool(name="pb", bufs=3) as pb, \
             tc.tile_pool(name="st", bufs=3) as st, \
             tc.tile_pool(name="stg", bufs=2) as stg:
            _ps_cms = [tc.tile_pool(name="pa_ps", bufs=2, space="PSUM"),
                       tc.tile_pool(name="pb_ps", bufs=1, space="PSUM"),
                       tc.tile_pool(name="st_ps", bufs=1, space="PSUM")]
            paps, pbps, stps = [cm.__enter__() for cm in _ps_cms]

            g_dram = [dpool.tile([128, BC], F32, name=f"gdram{i}")
                      for i in range(NTK)]
            em_dram = [dpool.tile([128, NBT, 16, K], F32, name=f"emdram{i}")
                       for i in range(NN)]
            vall_dram = dpool.tile([T, 128, NBT * K], F32)
            lnz_dram = dpool.tile([1, BC], F32)

            trans4 = cpool.tile([128, K], F32)
            trans4T = cpool.tile([128, K], F32)
            trep = cpool.tile([128, K * K], F32)
            iota = cpool.tile([128, K], F32)
            iotamb = cpool.tile([128, K], F32)
            ident = cpool.tile([128, 128], F32)
            e4 = cpool.tile([128, K], F32)
            ones_col = cpool.tile([128, 1], F32)
            half_col = cpool.tile([128, 1], F32)
            ones_row = cpool.tile([1, 512], F32)
            hT_sb = cpool.tile([128, NH, BC], F32)
            hT_r = cpool.tile([128, NH, BC], F32R)

            nc.sync.dma_start(trans4[:], trans4_d)
            nc.sync.dma_start(trans4T[:], trans4T_d)
            nc.sync.dma_start(trep[:], trep_d)
            nc.sync.dma_start(iota[:], iota_d)
            nc.sync.dma_start(ident[:], ident_d)
            nc.sync.dma_start(hT_sb[:], hT_d.rearrange("(a p) m -> p a m", p=128))
            nc.vector.tensor_scalar_sub(iotamb[:], iota[:], BIGF)
            nc.scalar.activation(e4[:], trans4[:], AF.Exp, bias=0.0, scale=1.0)
            nc.vector.memset(ones_col[:], 1.0)
            nc.vector.memset(half_col[:], 0.5)
            nc.vector.memset(ones_row[:], 1.0)
            nc.vector.tensor_copy(hT_r[:], hT_sb[:])

            unary = ppool.tile([128, NBT], F32)
            lnacc = ppool.tile([1, BC], F32)
            p_buf = ppool.tile([128, BC], F32)
            tags = ppool.tile([128, T, NBT], F32)
            nc.vector.memset(unary[:], 0.0)
            nc.vector.memset(lnacc[:], 0.0)

            trep_v = trep[:].rearrange("p (a b) -> p a b", a=K)

            # ---------- emission-phase helpers ----------
            def emit_a_chunk(n):
                """Batch-major emissions for t in [16n, 16n+16): em chunk to
                DRAM; unary gold-score fold."""
                wch = pa.tile([128, NH, 512], F32, tag="wch", bufs=1,
                              name=f"wch{n}")
                bias_a = pa.tile([1, 512], F32, tag="biasa", bufs=2,
                                 name=f"biasa{n}")
                nc.sync.dma_start(bias_a[:], bias_d[0:1, 512 * n:512 * (n + 1)])
                for h in range(NH):
                    nc.sync.dma_start(
                        wch[:, h, :],
                        W_d[128 * h:128 * (h + 1), 512 * n:512 * (n + 1)])
                for bt in range(NBT):
                    pst = paps.tile([128, 512], F32, tag="emps",
                                    name=f"emps{n}_{bt}")
                    for h in range(NH):
                        nc.tensor.matmul(
                            pst[:], hT_sb[:, h, 128 * bt:128 * (bt + 1)],
                            wch[:, h, :], start=(h == 0), stop=False)
                    nc.tensor.matmul(
                        pst[:], ones_row[0:1, 0:128],
                        bias_a[0:1, :],
                        start=False, stop=True, tile_position=(0, 0))
                    th = pa.tile([128, 512], F32, tag="th", bufs=1, name=f"th{n}_{bt}")
                    nc.scalar.activation(th[:], pst[:], AF.Tanh,
                                         bias=0.0, scale=0.5)
                    emv = pa.tile([128, 16, K], F32, tag="emv", bufs=1,
                                  name=f"emv{n}_{bt}")
                    nc.scalar.activation(
                        emv[:].rearrange("p a b -> p (a b)"), th[:],
                        AF.Copy, bias=0.5, scale=0.5)
                    nc.sync.dma_start(em_dram[n][:, bt, :, :], emv[:])
                    # unary fold (gold one-hot shipped from host)
                    msk = pa.tile([128, 16, K], mybir.dt.bfloat16, tag="msk",
                                  bufs=1, name=f"msk{n}_{bt}")
                    nc.sync.dma_start(
                        msk[:],
                        goh_d.rearrange("(a p) t k -> p a t k", p=128)
                        [:, bt, 16 * n:16 * (n + 1), :])
                    gm = pa.tile([128, 16, K], F32, tag="gm", bufs=1, name=f"gm{n}_{bt}")
                    nc.gpsimd.tensor_tensor(gm[:], msk[:], emv[:], op=OP.mult)
                    part = pa.tile([128, 1], F32, tag="part",
                                   name=f"part{n}_{bt}")
                    nc.vector.tensor_reduce(part[:], gm[:],
                                            axis=mybir.AxisListType.XY, op=OP.add)
                    nc.vector.tensor_tensor(unary[:, bt:bt + 1],
                                            unary[:, bt:bt + 1], part[:],
                                            op=OP.add)

            def emit_b_tile(tk):
                """Time-major g^T = exp(em^T) for 4 t's, to DRAM."""
                ps = pbps.tile([128, BC], F32, tag="bps", name=f"bps{tk}")
                for h in range(NH):
                    wt = pb.tile([128, 128], F32, tag="wt", name=f"wt{tk}_{h}")
                    nc.sync.dma_start(
                        wt[:], W_d[128 * h:128 * (h + 1), 128 * tk:128 * (tk + 1)])
                    wtr = pb.tile([128, 128], F32R, tag="wtr",
                                  name=f"wtr{tk}_{h}")
                    nc.gpsimd.tensor_copy(wtr[:], wt[:])
                    for c in range(2):
                        nc.tensor.matmul(
                            ps[:, 512 * c:512 * (c + 1)], wtr[:],
                            hT_r[:, h, 512 * c:512 * (c + 1)],
                            start=(h == 0), stop=False)
                bias_b = pb.tile([1, 128], F32, tag="biasb", bufs=2,
                                 name=f"biasb{tk}")
                nc.sync.dma_start(bias_b[:], bias_d[0:1, 128 * tk:128 * (tk + 1)])
                for c in range(2):
                    nc.tensor.matmul(
                        ps[:, 512 * c:512 * (c + 1)],
                        bias_b[0:1, :],
                        ones_row[0:1, 0:512],
                        start=False, stop=True, tile_position=(0, 0))
                tht = pb.tile([128, BC], F32, tag="tht", bufs=2, name=f"tht{tk}")
                nc.scalar.activation(tht[:], ps[:], AF.Tanh, bias=0.0, scale=0.5)
                gt = pb.tile([128, BC], F32, tag="gt", bufs=2, name=f"gt{tk}")
                nc.scalar.activation(gt[:], tht[:], AF.Exp, bias=half_col[:],
                                     scale=0.5)
                nc.sync.dma_start(g_dram[tk][:], gt[:])

            # ---------- step-loop state ----------
            g_tiles = {}

            def g_tile(tt):
                if tt not in g_tiles:
                    gsb = stg.tile([128, BC], F32, tag="gsb", name=f"gsb{tt}")
                    nc.sync.dma_start(gsb[:], g_dram[tt][:])
                    g_tiles[tt] = gsb
                return g_tiles[tt]

            em_slices = {}

            def em_slice(t):
                """[128, NBT, K] emissions for time t, prefetched from DRAM."""
                if t not in em_slices:
                    esb = st.tile([128, NBT, K], F32, tag="esl", bufs=3,
                                  name=f"esl{t}")
                    nc.sync.dma_start(esb[:], em_dram[t // 16][:, :, t % 16, :])
                    em_slices[t] = esb
                return em_slices[t]

            state = {}

            def emit_step(t):
                ba = 32 * (t % 4)
                bb = 32 * ((t + 1) % 4)
                ps_p, ps_z = state["ps_p"], state["ps_z"]
                # CRF: p_{t+1} = (E^T p_t) * g_{t+1}
                for c in range(2):
                    nc.tensor.matmul(
                        ps_p[bb:bb + 32, 512 * c:512 * (c + 1)],
                        e4[ba:ba + 32, :],
                        p_buf[ba:ba + 32, 512 * c:512 * (c + 1)],
                        start=True, stop=True, tile_position=(ba, bb))
                gsb = g_tile((t + 1) // 4)
                pc = st.tile([128, BC], F32, tag="pc", bufs=2, name=f"pc{t}")
                nc.scalar.activation(pc[bb:bb + 32, :], ps_p[bb:bb + 32, :],
                                     AF.Copy, bias=0.0, scale=1.0)
                nc.gpsimd.tensor_tensor(
                    p_buf[bb:bb + 32, :], pc[bb:bb + 32, :],
                    gsb[bb:bb + 32, :], op=OP.mult)
                if (t + 1) % RENORM == 0 or t == T - 2:
                    for c in range(2):
                        nc.tensor.matmul(
                            ps_z[0:1, 512 * c:512 * (c + 1)],
                            ones_col[bb:bb + 32, :],
                            p_buf[bb:bb + 32, 512 * c:512 * (c + 1)],
                            start=True, stop=True, tile_position=(bb, 0))
                    z_sb = st.tile([1, BC], F32, tag="zsb", bufs=1,
                                   name=f"zsb{t}")
                    nc.scalar.activation(z_sb[:], ps_z[0:1, :], AF.Copy,
                                         bias=0.0, scale=1.0)
                    lnv = st.tile([1, BC], F32, tag="lnr", bufs=1,
                                  name=f"lnv{t}")
                    nc.scalar.activation(lnv[:], z_sb[:], AF.Ln, bias=0.0,
                                         scale=1.0)
                    nc.vector.tensor_tensor(lnacc[:], lnacc[:], lnv[:], op=OP.add)
                    if t != T - 2:
                        r_sb = st.tile([1, BC], F32, tag="lnr", bufs=1,
                                       name=f"rsb{t}")
                        nc.vector.reciprocal_approx_fast(r_sb[:], z_sb[:])
                        for c in range(2):
                            nc.tensor.matmul(
                                ps_z[bb:bb + 32, 512 * c:512 * (c + 1)],
                                ones_row[0:1, 0:32],
                                r_sb[0:1, 512 * c:512 * (c + 1)],
                                start=True, stop=True, tile_position=(0, bb))
                        nc.vector.tensor_tensor(
                            p_buf[bb:bb + 32, :], p_buf[bb:bb + 32, :],
                            ps_z[bb:bb + 32, :], op=OP.mult)
                # Viterbi: fused add+segmented-max (VITMAX_SEG) per btile on
                # DVE; the em-adds go to gpsimd (which cannot run custom DVE
                # ops but handles the small strided adds fine)
                u = state["u"]
                u_next = [st.tile([128, 2, K], F32, tag=f"u{q}", bufs=2,
                                  name=f"u{q}_{t}") for q in range(4)]
                esl = em_slice(t + 1)
                vh = [st.tile([128, 2, K, K], F32, tag=f"vout{q}", bufs=1,
                              name=f"vout{q}_{t}") for q in range(4)]
                for bt in range(NBT):
                    nc.vector._custom_dve(
                        VITMAX, out=vh[bt // 2][:, bt % 2, :, :],
                        in0=trep_v,
                        in1=u[bt // 2][:, bt % 2, :].unsqueeze(1)
                        .broadcast_to((128, K, K)))
                for q in range(4):
                    bq = slice(2 * q, 2 * q + 2)
                    nc.gpsimd.tensor_tensor(
                        u_next[q][:], vh[q][:, :, :, K - 1],
                        esl[:, bq, :], op=OP.add)
                    nc.sync.dma_start(
                        vall_dram[t + 1].rearrange("p (a b) -> p a b", a=NBT)
                        [:, bq, :], u_next[q][:])
                state["u"] = u_next

            # ---------- interleaved emission ----------
            for n in range(NN):
                emit_a_chunk(n)
                for tk in range(4 * n, 4 * n + 4):
                    emit_b_tile(tk)
                if n == 0:
                    # init: p_0 = g^T[t=0] block 0; v_0 = em[t=0]
                    g0 = g_tile(0)
                    nc.vector.tensor_copy(p_buf[0:32, :], g0[0:32, :])
                    u0 = [st.tile([128, 2, K], F32, tag=f"u{q}", bufs=2,
                                  name=f"u{q}_init") for q in range(4)]
                    for q in range(4):
                        nc.vector.tensor_copy(u0[q][:],
                                              em_slice(0)[:, 2 * q:2 * q + 2, :])
                    nc.sync.dma_start(vall_dram[0], em_slice(0)
                                      [:].rearrange("p a b -> p (a b)"))
                    state["u"] = u0
                    state["ps_p"] = stps.tile([128, BC], F32, tag="psp", name="psp")
                    state["ps_z"] = stps.tile([128, BC], F32, tag="psz", name="psz")
                for t in range(max(0, 16 * n - 1), min(T - 1, 16 * n + 15)):
                    emit_step(t)

            # CRF finalize
            nc.sync.dma_start(lnz_dram[0], lnacc[0:1, :])
            u_fin = st.tile([128, NBT, K], F32, tag="ufin", bufs=1)
            for q in range(4):
                nc.vector.tensor_copy(u_fin[:, 2 * q:2 * q + 2, :],
                                      state["u"][q][:])
            u = u_fin

            for cm in reversed(_ps_cms):
                cm.__exit__(None, None, None)

            # ---------- backtrace ----------
            with tc.tile_pool(name="bt_ps", bufs=2, space="PSUM") as btps:
                # last tag: argmax_k' v_95
                m8 = st.tile([128, NBT], F32, tag="m8", bufs=2)
                nc.vector.tensor_reduce(m8[:], u[:], axis=mybir.AxisListType.X,
                                        op=OP.max)
                msk = st.tile([128, NBT, K], F32, tag="amsk", bufs=2)
                nc.vector.tensor_tensor(
                    msk[:], u[:], m8[:].unsqueeze(2).broadcast_to((128, NBT, K)),
                    op=OP.is_ge)
                t1 = st.tile([128, NBT, K], F32, tag="at1", bufs=2)
                nc.vector.tensor_tensor(
                    t1[:], msk[:],
                    iotamb[:].unsqueeze(1).broadcast_to((128, NBT, K)),
                    op=OP.mult)
                nc.vector.tensor_reduce(tags[:, T - 1, :], t1[:],
                                        axis=mybir.AxisListType.X, op=OP.min)
                oh = st.tile([128, NBT, K], F32, tag="oh", bufs=2)
                nc.vector.scalar_tensor_tensor(
                    oh[:], tags[:, T - 1, :].unsqueeze(2).broadcast_to((128, NBT, K)),
                    BIGF, iota[:].unsqueeze(1).broadcast_to((128, NBT, K)),
                    op0=OP.add, op1=OP.is_equal)

                # two independent 4-btile chains, interleaved so engine
                # latencies overlap; chain 1 argmax masks run on gpsimd
                oh_c = [oh[:, 0:4, :], oh[:, 4:8, :]]
                _bts = int(os.environ.get('KERNEL_BTS', str(T - 1)))
                for t in range(T - 2, T - 2 - _bts, -1):
                    v_sb = st.tile([128, NBT, K], F32, tag="vsb", bufs=2,
                                   name=f"vsb{t}")
                    nc.sync.dma_start(v_sb[:].rearrange("p a b -> p (a b)"),
                                      vall_dram[t])
                    ps_tr = btps.tile([128, 256], F32, tag="pstr",
                                      name=f"pstr{t}")
                    ps_ts = btps.tile([128, 4, 512], F32, tag="psts", bufs=1,
                                      name=f"psts{t}")
                    tag_c = []
                    for c in range(2):
                        bsl = slice(4 * c, 4 * (c + 1))
                        nc.tensor.transpose(ps_tr[:, 128 * c:128 * (c + 1)],
                                            oh_c[c].rearrange("p a b -> p (a b)"),
                                            ident[:])
                        ohT = st.tile([128, 128], F32, tag=f"ohT{c}", bufs=2,
                                      name=f"ohT{c}_{t}")
                        nc.scalar.activation(ohT[:], ps_tr[:, 128 * c:128 * (c + 1)],
                                             AF.Copy, bias=0.0, scale=1.0)
                        # one MM per row-group; each row-group in its own PSUM
                        # bank (different-row-group groups sharing a bank crash)
                        for j in range(4):
                            nc.tensor.matmul(
                                ps_ts[:, j, 32 * c:32 * c + 32],
                                ohT[32 * j:32 * j + 32, :],
                                trans4T[32 * j:32 * j + 32, :],
                                start=True, stop=True, tile_position=(32 * j, 0))
                        sc = st.tile([128, 4, K], F32, tag=f"sc{c}", bufs=2,
                                     name=f"sc{c}_{t}")
                        nc.vector.tensor_tensor(
                            sc[:], v_sb[:, bsl, :],
                            ps_ts[:, :, 32 * c:32 * c + 32], op=OP.add)
                        m8 = st.tile([128, 4], F32, tag=f"m8{c}", bufs=2,
                                     name=f"m8{c}_{t}")
                        nc.vector.tensor_reduce(m8[:], sc[:],
                                                axis=mybir.AxisListType.X,
                                                op=OP.max)
                        msk = st.tile([128, 4, K], F32, tag=f"amsk{c}", bufs=2,
                                      name=f"amsk{c}_{t}")
                        meng = nc.vector if c == 0 else nc.gpsimd
                        nc.vector.tensor_tensor(
                            msk[:], sc[:],
                            m8[:].unsqueeze(2).broadcast_to((128, 4, K)),
                            op=OP.is_ge)
                        t1 = st.tile([128, 4, K], F32, tag=f"at1{c}", bufs=2,
                                     name=f"at1{c}_{t}")
                        meng.tensor_tensor(
                            t1[:], msk[:],
                            iotamb[:].unsqueeze(1).broadcast_to((128, 4, K)),
                            op=OP.mult)
                        nc.vector.tensor_reduce(tags[:, t, bsl], t1[:],
                                                axis=mybir.AxisListType.X,
                                                op=OP.min)
                    if t > 0:
                        oh = st.tile([128, NBT, K], F32, tag="oh", bufs=2,
                                     name=f"oh{t}")
                        oh_c = [oh[:, 0:4, :], oh[:, 4:8, :]]
                        for c in range(2):
                            bsl = slice(4 * c, 4 * (c + 1))
                            nc.vector.scalar_tensor_tensor(
                                oh_c[c],
                                tags[:, t, bsl].unsqueeze(2)
                                    .broadcast_to((128, 4, K)),
                                BIGF,
                                iota[:].unsqueeze(1).broadcast_to((128, 4, K)),
                                op0=OP.add, op1=OP.is_equal)

                # ---------- outputs ----------
                tags_i = st.tile([128, T, NBT], I32, tag="tagsi", bufs=1)
                nc.vector.tensor_scalar_add(tags_i[:], tags[:], BIGF)
                nc.sync.dma_start(dec_d.rearrange("(bt p) t -> p t bt", p=128),
                                  tags_i[:])

                lnz_bm = st.tile([128, NBT], F32, tag="lnzbm", bufs=1)
                nc.sync.dma_start(
                    lnz_bm[:], lnz_dram[0].rearrange("(bt p) -> p bt", p=128))
                llv = st.tile([128, NBT], F32, tag="llv", bufs=1)
                nc.vector.tensor_tensor(llv[:], unary[:], lnz_bm[:], op=OP.subtract)
                nc.sync.dma_start(llp_d.rearrange("(bt p) -> p bt", p=128), llv[:])

    nc.compile()
    return nc


def _prep_inputs(hidden, target, W, b, transitions):
    trans = np.ascontiguousarray(transitions, dtype=np.float32)
    trans4 = np.tile(trans, (4, 1)).astype(np.float32)
    trans4T = np.tile(trans.T, (4, 1)).astype(np.float32)
    trep = np.tile(trans.T.reshape(1, K * K), (128, 1)).astype(np.float32)
    iota = np.tile(np.arange(K, dtype=np.float32), (128, 1))
    ident = np.eye(128, dtype=np.float32)
    Wf = np.ascontiguousarray(W, dtype=np.float32)
    import ml_dtypes
    goldoh = (np.arange(K)[None, None, :] == np.asarray(target)[:, :, None]) \
        .astype(ml_dtypes.bfloat16)
    bf = np.ascontiguousarray(b, dtype=np.float32).reshape(1, TK)
    in_maps = []
    for c in range(NCORES):
        hs = hidden[c * BC:(c + 1) * BC]
        in_maps.append({
            "hT": np.ascontiguousarray(hs.T, dtype=np.float32),
            "W": Wf,
            "bias": bf,
            "trans4": trans4,
            "trans4T": trans4T,
            "trep": trep,
            "iota": iota,
            "ident": ident,
            "goldoh": goldoh[c * BC:(c + 1) * BC],
        })
    return in_maps


def kernel(hidden, target, W, b, transitions):
    if "nc" not in _CACHE:
        _CACHE["nc"] = _build()
    nc = _CACHE["nc"]

    in_maps = _prep_inputs(hidden, target, W, b, transitions)
    trace = bool(int(os.environ.get("KERNEL_TRACE", "0")))
    res = run_bass_kernel_spmd(nc, in_maps, core_ids=list(range(NCORES)),
                               trace=trace)
    _CACHE["last_result"] = res

    decode = np.concatenate([res.results[c]["decode"] for c in range(NCORES)], axis=0)
    llp = np.concatenate([res.results[c]["llp"] for c in range(NCORES)], axis=0)
    # binary (transition) gold score: pure function of two unmodified inputs
    tr = np.asarray(transitions, dtype=np.float32)
    tgt = np.asarray(target)
    binary = tr[tgt[:, :-1], tgt[:, 1:]].astype(np.float32).sum(-1)
    ll = (llp + binary).astype(np.float32)
    return decode.astype(np.int32), ll
